# revision 40
# baseline (speedup 1.0000x reference)
"""Trainium2 Bass kernel for ExemplarGNN2AdjModel (gnn_message_passing).

Math:
  h  = relu(relu(x@W1+b1)@W2+b2)                      # [512,128] node encoder
  scores[i,j] = Wp2 . relu(Wp1a.h_i + Wp1b.h_j + Wp1c.|h_i-h_j| + bp1) + bp2

Device algorithm (per core, SPMD over 8 cores; core c handles 64 rows of i):
  - Each core receives x pre-rolled by c*64 rows and pre-transposed (xT), so the
    identical program computes rows [c*64, c*64+64) in its local (rolled) node
    order; the host un-rolls the output columns afterwards.
  - |h_i-h_j| = h_i + h_j - 2*min(h_i,h_j): the h_i term is folded into the
    per-i bias matrix (wp1a += w3), the h_j term into the B matmul
    (w2p += w3), and the per-pair part is -2*w3^T min(h_i, h_j) (abs is not
    ISA-encodable as a DVE tensor_scalar op; min is).
  - Encoder runs on-device in bf16 (all 512 nodes, replicated per core),
    fp32 PSUM accumulation, fp32 biases.  A2 = wp1a^T h + bp1 precomputed once.
  - Per row i (64 iters):
      d_i  = min(h, h_i)               (DVE tensor_scalar, pipelined 1 ahead)
      P    = w2p^T h + w3^T d_i        (accumulating PE matmuls into PSUM)
      hid  = relu(P + A2[:,i])         (split: ACT cols [0:SPLIT), DVE rest)
      out[i,:] += emb[:,i,:]^T hid     (PE matmul, Wp2 embedded in col i of a
                                        [128,64] zero matrix -> accumulates row
                                        i of a [64,512] PSUM tile; deferred 2
                                        iterations so the in-order PE never
                                        waits on the relus)
  - Dummy matmuls/activation at kernel start keep the PE busy during the input
    DMA phase so the HAM clock-gate warms up (2.4 GHz) before the real work.
  - out rows + bp2 -> DMA out.
"""

import numpy as np
import ml_dtypes

B = 512
IN_DIM = 595
HID = 128
NCORES = 8
RPC = B // NCORES  # rows per core = 64
SPLIT = 416  # relu column split: ACT does [0:SPLIT), DVE does [SPLIT:B)
N_WARM_MM = 6  # dummy matmuls to warm the PE HAM clock during input DMAs
DEFER = 4  # iterations between producing hid(i) and its out-row matmul

# in_dim k-tiles for the first encoder matmul (contraction over 595)
KT = [(0, 128), (128, 256), (256, 384), (384, 512), (512, 595)]

_PROGRAM_CACHE = {}


def _build_program():
    import concourse.mybir as mybir
    import concourse.tile as tile
    from concourse import bacc

    f32 = mybir.dt.float32
    bf16 = mybir.dt.bfloat16
    Act = mybir.ActivationFunctionType
    Alu = mybir.AluOpType

    nc = bacc.Bacc("TRN2", target_bir_lowering=False)

    NKT = len(KT)
    # Consolidated inputs (few big DMAs: the 8 DMA-completion semaphore lanes
    # are shared round-robin across all transfers, so many small DMAs create
    # false cross-DMA serialization):
    #   xt  padded to [5*128, 512] -> viewed [128, 5*512] (k-tiles on free dim)
    #   w1  padded to [5*128, 128] -> viewed [128, 5*128]
    #   wpack = [w2 | wp1a | w2p | w3]   [128, 4*128] bf16
    #   bias  = [b1 | b2 | bp1 | bp2col] [128, 4] f32
    xt_d = nc.dram_tensor("xtp", [HID, NKT * B], bf16, kind="ExternalInput")
    w1_d = nc.dram_tensor("w1p", [HID, NKT * HID], bf16, kind="ExternalInput")
    wpack_d = nc.dram_tensor("wpack", [HID, 4 * HID], bf16, kind="ExternalInput")
    bias_d = nc.dram_tensor("biases", [HID, 4], f32, kind="ExternalInput")
    emb_d = nc.dram_tensor("emb", [HID, RPC * RPC], bf16, kind="ExternalInput")
    out_d = nc.dram_tensor("out", [RPC, B], f32, kind="ExternalOutput")

    with tile.TileContext(nc) as tc:
        with (
            tc.tile_pool(name="consts", bufs=1) as consts,
            tc.tile_pool(name="setup", bufs=1) as setup,
            tc.tile_pool(name="work", bufs=9) as work,
        ):
            from contextlib import ExitStack
            setup_psum = ExitStack()
            penc = setup_psum.enter_context(
                tc.tile_pool(name="penc", bufs=1, space="PSUM")
            )
            pwarm = setup_psum.enter_context(
                tc.tile_pool(name="pwarm", bufs=1, space="PSUM")
            )
            # ---- PE warm-up: dummy matmuls on scratch keep the PE busy while
            # the input DMAs land, so HAM unthrottles the clock to 2.4 GHz and
            # the ACT table / IRAM loads happen off the critical path.
            scratch = setup.tile([HID, B], bf16)
            nc.vector.memset(scratch, 0.0)
            scratch1 = setup.tile([HID, 1], f32)
            nc.scalar.activation(scratch1, scratch[:, 0:1], Act.Relu)
            warmp = pwarm.tile([HID, B], f32)

            def warm_mm(n):
                # dummy matmuls with no data deps: keep the in-order PE busy
                # across gaps where it would otherwise idle long enough for
                # HAM to re-throttle the clock
                for _ in range(n):
                    nc.tensor.matmul(
                        warmp, lhsT=scratch[:, 0:HID], rhs=scratch,
                        start=True, stop=True, skip_group_check=True,
                    )

            warm_mm(N_WARM_MM)

            # ---- input loads: 5 consolidated DMAs on two HWDGE rings
            xt_all = consts.tile([HID, NKT * B], bf16)
            nc.sync.dma_start(out=xt_all, in_=xt_d[:, :])
            w1_all = consts.tile([HID, NKT * HID], bf16)
            nc.scalar.dma_start(out=w1_all, in_=w1_d[:, :])
            wpack = consts.tile([HID, 4 * HID], bf16)
            nc.scalar.dma_start(out=wpack, in_=wpack_d[:, :])
            biases = consts.tile([HID, 4], f32)
            nc.scalar.dma_start(out=biases, in_=bias_d[:, :])
            emb_sb = consts.tile([HID, RPC * RPC], bf16)
            nc.scalar.dma_start(out=emb_sb, in_=emb_d[:, :])

            xt_sb = [xt_all[:, k * B : (k + 1) * B] for k in range(NKT)]
            w1_sb = [w1_all[:, k * HID : (k + 1) * HID] for k in range(NKT)]
            w2_sb = wpack[:, 0 * HID : 1 * HID]
            wp1a_sb = wpack[:, 1 * HID : 2 * HID]
            w2p_sb = wpack[:, 2 * HID : 3 * HID]
            w3_sb = wpack[:, 3 * HID : 4 * HID]
            b1_sb = biases[:, 0:1]
            b2_sb = biases[:, 1:2]
            bp1_sb = biases[:, 2:3]
            bp2_sb = biases[0:RPC, 3:4]

            # ---- encoder: h1 = relu(W1^T xT + b1), hT = relu(W2^T h1 + b2) ----
            h1p = penc.tile([HID, B], f32, name="encp", tag="encp")
            for k in range(len(KT)):
                nc.tensor.matmul(
                    h1p, lhsT=w1_sb[k], rhs=xt_sb[k],
                    start=(k == 0), stop=(k == len(KT) - 1),
                )
            warm_mm(2)  # bridge PE over relu1
            h1bf = setup.tile([HID, B], bf16)
            nc.scalar.activation(h1bf, h1p, Act.Relu, bias=b1_sb)

            h2p = penc.tile([HID, B], f32, name="encp2", tag="encp")
            nc.tensor.matmul(h2p, lhsT=w2_sb, rhs=h1bf, start=True, stop=True)
            warm_mm(2)  # bridge PE over relu2
            # hbf (bf16, ACT) and hT (fp32, DVE) are produced in parallel from
            # the same PSUM tile; hT fp32 is needed only as the per-row scalar
            # operand of the min (tensor_scalar scalars must be fp32)
            hbf = setup.tile([HID, B], bf16)
            nc.scalar.activation(hbf, h2p, Act.Relu, bias=b2_sb)
            hT = setup.tile([HID, B], f32)
            nc.vector.tensor_scalar(hT, h2p, b2_sb, 0.0, Alu.add, Alu.max)

            # first two mins go ahead of the a2 add on the DVE queue: C(0)
            # gates the loop pipeline earlier than relu(0) needs a2
            def emit_min(j, dtiles):
                if j in dtiles or j >= RPC:
                    return
                d = work.tile([HID, B], bf16, name="dtile")
                nc.vector.tensor_scalar(
                    d, hbf, hT[:, j : j + 1], None, Alu.min
                )
                dtiles[j] = d

            dtiles = {}
            emit_min(0, dtiles)
            emit_min(1, dtiles)

            # ---- A2 = wp1a^T h + bp1  (per-i relu bias columns) ----
            a2p = penc.tile([HID, B], f32, name="encp3", tag="encp")
            nc.tensor.matmul(a2p, lhsT=wp1a_sb, rhs=hbf, start=True, stop=True)
            a2 = setup.tile([HID, B], f32)
            nc.vector.tensor_scalar(a2, a2p, bp1_sb, None, Alu.add)
            emit_min(2, dtiles)
            emit_min(3, dtiles)
            warm_mm(2)  # bridge PE over the a2 add + first mins

            # ---- pairwise main loop over this core's 64 rows ----
            # release the setup PSUM banks so the pair pool can go deeper
            setup_psum.close()
            ppair = tc.alloc_tile_pool(name="ppair", bufs=6, space="PSUM")
            pout = tc.alloc_tile_pool(name="pout", bufs=1, space="PSUM")
            outp = pout.tile([RPC, B], f32)
            pending = {}

            def emit_out(j):
                hid_j = pending.pop(j)
                nc.tensor.matmul(
                    outp, lhsT=emb_sb[:, j * RPC : (j + 1) * RPC], rhs=hid_j,
                    start=(j == 0), stop=(j == RPC - 1),
                    skip_group_check=True,
                )

            # Process rows in groups of G sharing the w2p/w3 weight loads: the
            # PE has only two weight buffers, so with three stationaries per
            # row one LDWEIGHTS per row cannot be hidden behind a matmul.
            # Grouping amortizes the w2p/w3 loads and lets loads overlap MMs.
            G = 2
            for g in range((RPC + G - 1) // G):
                ils = [i for i in range(G * g, min(G * g + G, RPC))]
                for il in ils:
                    emit_min(il + G, dtiles)
                pps = []
                for il in ils:
                    pp = ppair.tile([HID, B], f32, name="pp")
                    nc.tensor.matmul(
                        pp, lhsT=w2p_sb, rhs=hbf,
                        start=True, stop=False, skip_group_check=True,
                    )
                    pps.append(pp)
                for il, pp in zip(ils, pps):
                    nc.tensor.matmul(
                        pp, lhsT=w3_sb, rhs=dtiles.pop(il),
                        start=False, stop=True, skip_group_check=True,
                    )
                for il, pp in zip(ils, pps):
                    hid = work.tile([HID, B], bf16, name="hid")
                    nc.scalar.activation(
                        hid[:, 0:SPLIT], pp[:, 0:SPLIT], Act.Relu,
                        bias=a2[:, il : il + 1],
                    )
                    nc.vector.tensor_scalar(
                        hid[:, SPLIT:B], pp[:, SPLIT:B],
                        a2[:, il : il + 1], 0.0, Alu.add, Alu.max,
                    )
                    pending[il] = hid
                    if il >= DEFER:
                        emit_out(il - DEFER)
            for j in range(RPC - DEFER, RPC):
                emit_out(j)

            outs = setup.tile([RPC, B], f32)
            nc.vector.tensor_scalar(outs, outp, bp2_sb, None, Alu.add)
            nc.sync.dma_start(out=out_d[:, :], in_=outs)
            pout.release()
            ppair.release()

    nc.finalize()
    return nc


def _get_program():
    if "nc" not in _PROGRAM_CACHE:
        _PROGRAM_CACHE["nc"] = _build_program()
    return _PROGRAM_CACHE["nc"]


def _make_in_maps(x, W1, b1, W2, b2, Wp1, bp1, Wp2, bp2):
    bf16 = ml_dtypes.bfloat16
    f32 = np.float32
    x = np.asarray(x, dtype=f32)
    W1 = np.asarray(W1, dtype=f32)
    W2 = np.asarray(W2, dtype=f32)
    Wp1 = np.asarray(Wp1, dtype=f32)
    Wp2 = np.asarray(Wp2, dtype=f32).reshape(HID, 1)
    b1c = np.ascontiguousarray(np.asarray(b1, dtype=f32).reshape(HID, 1))
    b2c = np.ascontiguousarray(np.asarray(b2, dtype=f32).reshape(HID, 1))
    bp1c = np.ascontiguousarray(np.asarray(bp1, dtype=f32).reshape(HID, 1))
    bp2c = np.full((RPC, 1), np.asarray(bp2, dtype=f32).reshape(-1)[0], dtype=f32)

    # |h_i - h_j| = h_i + h_j - 2*min(h_i, h_j) folds (see module docstring)
    w3f = Wp1[2 * HID : 3 * HID, :]
    wp1a = Wp1[0:HID, :] + w3f
    w2p = Wp1[HID : 2 * HID, :] + w3f
    w3 = -2.0 * w3f

    NKT = len(KT)
    KPAD = NKT * HID  # 640: in_dim padded so every k-tile is 128 partitions

    # Wp2 embedded: emb[:, il, c] = Wp2[:,0] if c == il else 0
    emb = np.zeros((HID, RPC, RPC), dtype=f32)
    idx = np.arange(RPC)
    emb[:, idx, idx] = Wp2
    emb = np.ascontiguousarray(emb.reshape(HID, RPC * RPC)).astype(bf16)

    # packed weights [w2 | wp1a | w2p | w3] and biases [b1 | b2 | bp1 | bp2col]
    wpack = np.concatenate([W2, wp1a, w2p, w3], axis=1).astype(bf16)
    biases = np.zeros((HID, 4), dtype=f32)
    biases[:, 0:1] = b1c
    biases[:, 1:2] = b2c
    biases[:, 2:3] = bp1c
    biases[0:RPC, 3:4] = bp2c

    # w1 padded to [640, 128], viewed as [128, 5*128]
    w1_pad = np.zeros((KPAD, HID), dtype=f32)
    w1_pad[:IN_DIM] = np.asarray(W1, dtype=f32)
    w1p = np.ascontiguousarray(
        w1_pad.reshape(NKT, HID, HID).transpose(1, 0, 2).reshape(HID, NKT * HID)
    ).astype(bf16)

    shared = dict(w1p=w1p, wpack=wpack, biases=biases, emb=emb)
    in_maps = []
    for c in range(NCORES):
        xr = np.roll(x, -c * RPC, axis=0)
        xt_pad = np.zeros((KPAD, B), dtype=f32)
        xt_pad[:IN_DIM] = xr.T
        xtp = np.ascontiguousarray(
            xt_pad.reshape(NKT, HID, B).transpose(1, 0, 2).reshape(HID, NKT * B)
        ).astype(bf16)
        m = dict(shared)
        m["xtp"] = xtp
        in_maps.append(m)
    return in_maps


def _run(in_maps, trace=False):
    from concourse.bass_utils import run_bass_kernel_spmd

    nc = _get_program()
    return run_bass_kernel_spmd(
        nc, in_maps, core_ids=list(range(NCORES)), trace=trace
    )


def kernel(x, W1, b1, W2, b2, Wp1, bp1, Wp2, bp2):
    in_maps = _make_in_maps(x, W1, b1, W2, b2, Wp1, bp1, Wp2, bp2)
    res = _run(in_maps, trace=False)
    out = np.empty((B, B), dtype=np.float32)
    for c in range(NCORES):
        blk = np.asarray(res.results[c]["out"], dtype=np.float32)
        out[c * RPC : (c + 1) * RPC, :] = np.roll(blk, c * RPC, axis=1)
    return out


# revision 41
# speedup vs baseline: 1.0427x; 1.0427x over previous
"""Trainium2 Bass kernel for ExemplarGNN2AdjModel (gnn_message_passing).

Math:
  h  = relu(relu(x@W1+b1)@W2+b2)                      # [512,128] node encoder
  scores[i,j] = Wp2 . relu(Wp1a.h_i + Wp1b.h_j + Wp1c.|h_i-h_j| + bp1) + bp2

Device algorithm (per core, SPMD over 8 cores; core c handles 64 rows of i):
  - Each core receives x pre-rolled by c*64 rows and pre-transposed (xT), so the
    identical program computes rows [c*64, c*64+64) in its local (rolled) node
    order; the host un-rolls the output columns afterwards.
  - |h_i-h_j| = h_i + h_j - 2*min(h_i,h_j): the h_i term is folded into the
    per-i bias matrix (wp1a += w3), the h_j term into the B matmul
    (w2p += w3), and the per-pair part is -2*w3^T min(h_i, h_j) (abs is not
    ISA-encodable as a DVE tensor_scalar op; min is).
  - Encoder runs on-device in bf16 (all 512 nodes, replicated per core),
    fp32 PSUM accumulation, fp32 biases.  A2 = wp1a^T h + bp1 precomputed once.
  - Per row i (64 iters):
      d_i  = min(h, h_i)               (DVE tensor_scalar, pipelined 1 ahead)
      P    = w2p^T h + w3^T d_i        (accumulating PE matmuls into PSUM)
      hid  = relu(P + A2[:,i])         (split: ACT cols [0:SPLIT), DVE rest)
      out[i,:] += emb[:,i,:]^T hid     (PE matmul, Wp2 embedded in col i of a
                                        [128,64] zero matrix -> accumulates row
                                        i of a [64,512] PSUM tile; deferred 2
                                        iterations so the in-order PE never
                                        waits on the relus)
  - Dummy matmuls/activation at kernel start keep the PE busy during the input
    DMA phase so the HAM clock-gate warms up (2.4 GHz) before the real work.
  - out rows + bp2 -> DMA out.
"""

import numpy as np
import ml_dtypes

B = 512
IN_DIM = 595
HID = 128
NCORES = 8
RPC = B // NCORES  # rows per core = 64
SPLIT = 416  # relu column split: ACT does [0:SPLIT), DVE does [SPLIT:B)
N_WARM_MM = 6  # dummy matmuls to warm the PE HAM clock during input DMAs
DEFER = 4  # iterations between producing hid(i) and its out-row matmul

# in_dim k-tiles for the first encoder matmul (contraction over 595)
KT = [(0, 128), (128, 256), (256, 384), (384, 512), (512, 595)]

_PROGRAM_CACHE = {}


def _build_program():
    import concourse.mybir as mybir
    import concourse.tile as tile
    from concourse import bacc

    f32 = mybir.dt.float32
    bf16 = mybir.dt.bfloat16
    Act = mybir.ActivationFunctionType
    Alu = mybir.AluOpType

    nc = bacc.Bacc("TRN2", target_bir_lowering=False)

    NKT = len(KT)
    # Consolidated inputs (few big DMAs: the 8 DMA-completion semaphore lanes
    # are shared round-robin across all transfers, so many small DMAs create
    # false cross-DMA serialization):
    #   xt  padded to [5*128, 512] -> viewed [128, 5*512] (k-tiles on free dim)
    #   w1  padded to [5*128, 128] -> viewed [128, 5*128]
    #   wpack = [w2 | wp1a | w2p | w3]   [128, 4*128] bf16
    #   bias  = [b1 | b2 | bp1 | bp2col] [128, 4] f32
    xt_d = nc.dram_tensor("xtp", [HID, NKT * B], bf16, kind="ExternalInput")
    w1_d = nc.dram_tensor("w1p", [HID, NKT * HID], bf16, kind="ExternalInput")
    wpack_d = nc.dram_tensor("wpack", [HID, 4 * HID], bf16, kind="ExternalInput")
    bias_d = nc.dram_tensor("biases", [HID, 4], f32, kind="ExternalInput")
    emb_d = nc.dram_tensor("emb", [HID, RPC * RPC], bf16, kind="ExternalInput")
    out_d = nc.dram_tensor("out", [RPC, B], f32, kind="ExternalOutput")

    with tile.TileContext(nc) as tc:
        with (
            tc.tile_pool(name="consts", bufs=1) as consts,
            tc.tile_pool(name="setup", bufs=1) as setup,
            tc.tile_pool(name="work", bufs=9) as work,
            tc.tile_pool(name="penc", bufs=1, space="PSUM") as penc,
            tc.tile_pool(name="ppair", bufs=5, space="PSUM") as ppair,
            tc.tile_pool(name="pout", bufs=1, space="PSUM") as pout,
            tc.tile_pool(name="pwarm", bufs=1, space="PSUM") as pwarm,
        ):
            # ---- PE warm-up: dummy matmuls on scratch keep the PE busy while
            # the input DMAs land, so HAM unthrottles the clock to 2.4 GHz and
            # the ACT table / IRAM loads happen off the critical path.
            scratch = setup.tile([HID, B], bf16)
            nc.vector.memset(scratch, 0.0)
            scratch1 = setup.tile([HID, 1], f32)
            nc.scalar.activation(scratch1, scratch[:, 0:1], Act.Relu)
            warmp = pwarm.tile([HID, B], f32)

            def warm_mm(n):
                # dummy matmuls with no data deps: keep the in-order PE busy
                # across gaps where it would otherwise idle long enough for
                # HAM to re-throttle the clock
                for _ in range(n):
                    nc.tensor.matmul(
                        warmp, lhsT=scratch[:, 0:HID], rhs=scratch,
                        start=True, stop=True, skip_group_check=True,
                    )

            warm_mm(N_WARM_MM)

            # ---- input loads: 5 consolidated DMAs on two HWDGE rings
            xt_all = consts.tile([HID, NKT * B], bf16)
            nc.sync.dma_start(out=xt_all, in_=xt_d[:, :])
            w1_all = consts.tile([HID, NKT * HID], bf16)
            nc.scalar.dma_start(out=w1_all, in_=w1_d[:, :])
            wpack = consts.tile([HID, 4 * HID], bf16)
            nc.scalar.dma_start(out=wpack, in_=wpack_d[:, :])
            biases = consts.tile([HID, 4], f32)
            nc.scalar.dma_start(out=biases, in_=bias_d[:, :])
            emb_sb = consts.tile([HID, RPC * RPC], bf16)
            nc.scalar.dma_start(out=emb_sb, in_=emb_d[:, :])

            xt_sb = [xt_all[:, k * B : (k + 1) * B] for k in range(NKT)]
            w1_sb = [w1_all[:, k * HID : (k + 1) * HID] for k in range(NKT)]
            w2_sb = wpack[:, 0 * HID : 1 * HID]
            wp1a_sb = wpack[:, 1 * HID : 2 * HID]
            w2p_sb = wpack[:, 2 * HID : 3 * HID]
            w3_sb = wpack[:, 3 * HID : 4 * HID]
            b1_sb = biases[:, 0:1]
            b2_sb = biases[:, 1:2]
            bp1_sb = biases[:, 2:3]
            bp2_sb = biases[0:RPC, 3:4]

            # ---- encoder: h1 = relu(W1^T xT + b1), hT = relu(W2^T h1 + b2) ----
            h1p = penc.tile([HID, B], f32, name="encp", tag="encp")
            for k in range(len(KT)):
                nc.tensor.matmul(
                    h1p, lhsT=w1_sb[k], rhs=xt_sb[k],
                    start=(k == 0), stop=(k == len(KT) - 1),
                )
            warm_mm(2)  # bridge PE over relu1
            h1bf = setup.tile([HID, B], bf16)
            nc.scalar.activation(h1bf, h1p, Act.Relu, bias=b1_sb)

            h2p = penc.tile([HID, B], f32, name="encp2", tag="encp")
            nc.tensor.matmul(h2p, lhsT=w2_sb, rhs=h1bf, start=True, stop=True)
            warm_mm(2)  # bridge PE over relu2
            # hbf (bf16, ACT) and hT (fp32, DVE) are produced in parallel from
            # the same PSUM tile; hT fp32 is needed only as the per-row scalar
            # operand of the min (tensor_scalar scalars must be fp32)
            hbf = setup.tile([HID, B], bf16)
            nc.scalar.activation(hbf, h2p, Act.Relu, bias=b2_sb)
            hT = setup.tile([HID, B], f32)
            nc.vector.tensor_scalar(hT, h2p, b2_sb, 0.0, Alu.add, Alu.max)

            # first two mins go ahead of the a2 add on the DVE queue: C(0)
            # gates the loop pipeline earlier than relu(0) needs a2
            def emit_min(j, dtiles):
                if j in dtiles or j >= RPC:
                    return
                d = work.tile([HID, B], bf16, name="dtile")
                nc.vector.tensor_scalar(
                    d, hbf, hT[:, j : j + 1], None, Alu.min
                )
                dtiles[j] = d

            dtiles = {}
            emit_min(0, dtiles)
            emit_min(1, dtiles)

            # ---- A2 = wp1a^T h + bp1  (per-i relu bias columns) ----
            a2p = penc.tile([HID, B], f32, name="encp3", tag="encp")
            nc.tensor.matmul(a2p, lhsT=wp1a_sb, rhs=hbf, start=True, stop=True)
            a2 = setup.tile([HID, B], f32)
            nc.vector.tensor_scalar(a2, a2p, bp1_sb, None, Alu.add)
            emit_min(2, dtiles)
            emit_min(3, dtiles)
            warm_mm(2)  # bridge PE over the a2 add + first mins

            # ---- pairwise main loop over this core's 64 rows ----
            outp = pout.tile([RPC, B], f32)
            pending = {}

            def emit_out(j):
                hid_j = pending.pop(j)
                nc.tensor.matmul(
                    outp, lhsT=emb_sb[:, j * RPC : (j + 1) * RPC], rhs=hid_j,
                    start=(j == 0), stop=(j == RPC - 1),
                    skip_group_check=True,
                )

            # Process rows in groups of G sharing the w2p/w3 weight loads: the
            # PE has only two weight buffers, so with three stationaries per
            # row one LDWEIGHTS per row cannot be hidden behind a matmul.
            # Grouping amortizes the w2p/w3 loads and lets loads overlap MMs.
            G = 2
            for g in range((RPC + G - 1) // G):
                ils = [i for i in range(G * g, min(G * g + G, RPC))]
                for il in ils:
                    emit_min(il + G, dtiles)
                pps = []
                for il in ils:
                    pp = ppair.tile([HID, B], f32, name="pp")
                    nc.tensor.matmul(
                        pp, lhsT=w2p_sb, rhs=hbf,
                        start=True, stop=False, skip_group_check=True,
                    )
                    pps.append(pp)
                for il, pp in zip(ils, pps):
                    nc.tensor.matmul(
                        pp, lhsT=w3_sb, rhs=dtiles.pop(il),
                        start=False, stop=True, skip_group_check=True,
                    )
                for il, pp in zip(ils, pps):
                    hid = work.tile([HID, B], bf16, name="hid")
                    nc.scalar.activation(
                        hid[:, 0:SPLIT], pp[:, 0:SPLIT], Act.Relu,
                        bias=a2[:, il : il + 1],
                    )
                    nc.vector.tensor_scalar(
                        hid[:, SPLIT:B], pp[:, SPLIT:B],
                        a2[:, il : il + 1], 0.0, Alu.add, Alu.max,
                    )
                    pending[il] = hid
                    if il >= DEFER:
                        emit_out(il - DEFER)
            for j in range(RPC - DEFER, RPC):
                emit_out(j)

            outs = setup.tile([RPC, B], f32)
            nc.vector.tensor_scalar(outs, outp, bp2_sb, None, Alu.add)
            nc.sync.dma_start(out=out_d[:, :], in_=outs)

    nc.finalize()
    return nc


def _get_program():
    if "nc" not in _PROGRAM_CACHE:
        _PROGRAM_CACHE["nc"] = _build_program()
    return _PROGRAM_CACHE["nc"]


def _make_in_maps(x, W1, b1, W2, b2, Wp1, bp1, Wp2, bp2):
    bf16 = ml_dtypes.bfloat16
    f32 = np.float32
    x = np.asarray(x, dtype=f32)
    W1 = np.asarray(W1, dtype=f32)
    W2 = np.asarray(W2, dtype=f32)
    Wp1 = np.asarray(Wp1, dtype=f32)
    Wp2 = np.asarray(Wp2, dtype=f32).reshape(HID, 1)
    b1c = np.ascontiguousarray(np.asarray(b1, dtype=f32).reshape(HID, 1))
    b2c = np.ascontiguousarray(np.asarray(b2, dtype=f32).reshape(HID, 1))
    bp1c = np.ascontiguousarray(np.asarray(bp1, dtype=f32).reshape(HID, 1))
    bp2c = np.full((RPC, 1), np.asarray(bp2, dtype=f32).reshape(-1)[0], dtype=f32)

    # |h_i - h_j| = h_i + h_j - 2*min(h_i, h_j) folds (see module docstring)
    w3f = Wp1[2 * HID : 3 * HID, :]
    wp1a = Wp1[0:HID, :] + w3f
    w2p = Wp1[HID : 2 * HID, :] + w3f
    w3 = -2.0 * w3f

    NKT = len(KT)
    KPAD = NKT * HID  # 640: in_dim padded so every k-tile is 128 partitions

    # Wp2 embedded: emb[:, il, c] = Wp2[:,0] if c == il else 0
    emb = np.zeros((HID, RPC, RPC), dtype=f32)
    idx = np.arange(RPC)
    emb[:, idx, idx] = Wp2
    emb = np.ascontiguousarray(emb.reshape(HID, RPC * RPC)).astype(bf16)

    # packed weights [w2 | wp1a | w2p | w3] and biases [b1 | b2 | bp1 | bp2col]
    wpack = np.concatenate([W2, wp1a, w2p, w3], axis=1).astype(bf16)
    biases = np.zeros((HID, 4), dtype=f32)
    biases[:, 0:1] = b1c
    biases[:, 1:2] = b2c
    biases[:, 2:3] = bp1c
    biases[0:RPC, 3:4] = bp2c

    # w1 padded to [640, 128], viewed as [128, 5*128]
    w1_pad = np.zeros((KPAD, HID), dtype=f32)
    w1_pad[:IN_DIM] = np.asarray(W1, dtype=f32)
    w1p = np.ascontiguousarray(
        w1_pad.reshape(NKT, HID, HID).transpose(1, 0, 2).reshape(HID, NKT * HID)
    ).astype(bf16)

    shared = dict(w1p=w1p, wpack=wpack, biases=biases, emb=emb)
    in_maps = []
    for c in range(NCORES):
        xr = np.roll(x, -c * RPC, axis=0)
        xt_pad = np.zeros((KPAD, B), dtype=f32)
        xt_pad[:IN_DIM] = xr.T
        xtp = np.ascontiguousarray(
            xt_pad.reshape(NKT, HID, B).transpose(1, 0, 2).reshape(HID, NKT * B)
        ).astype(bf16)
        m = dict(shared)
        m["xtp"] = xtp
        in_maps.append(m)
    return in_maps


def _run(in_maps, trace=False):
    from concourse.bass_utils import run_bass_kernel_spmd

    nc = _get_program()
    return run_bass_kernel_spmd(
        nc, in_maps, core_ids=list(range(NCORES)), trace=trace
    )


def kernel(x, W1, b1, W2, b2, Wp1, bp1, Wp2, bp2):
    in_maps = _make_in_maps(x, W1, b1, W2, b2, Wp1, bp1, Wp2, bp2)
    res = _run(in_maps, trace=False)
    out = np.empty((B, B), dtype=np.float32)
    for c in range(NCORES):
        blk = np.asarray(res.results[c]["out"], dtype=np.float32)
        out[c * RPC : (c + 1) * RPC, :] = np.roll(blk, c * RPC, axis=1)
    return out
